# revision 1
# baseline (speedup 1.0000x reference)
"""Contrastive loss kernel for 8 Trainium2 NeuronCores.

Math (reference): normalize rows of input/target/hard_negative; logits =
[xn@tn.T, xn@hn.T]/TEMP with +1.0 added on the hard-negative diagonal;
loss = -mean(log_softmax(logits)[i, i]).

Equivalent: loss = mean_i( log(sum_c exp(logits[i, c])) - pos_diag_i ).

Sharding: 2x4 grid. Core (i, j) handles 2048 input rows (half i) against a
1024-row chunk of target/hard_negative. Per-core host-side row permutation
makes the diagonal land at identical local coordinates on every core (local
rows 0..511 <-> local cols 0..511), so one SPMD program serves all 8 cores.
Each core returns its partial sum-of-exp per row plus the pos-diagonal
values it owns; the host adds partials, takes log, and averages.
"""

import sys

sys.path.insert(0, "/opt/trn_rl_repo")

import numpy as np

import concourse.bass as bass
import concourse.tile as tile
from concourse import bacc, mybir
from concourse.masks import make_identity

N, D = 4096, 1024
TEMP = 0.05
SCALE = 1.0 / TEMP
HARD_NEG_WEIGHT = 1.0
EPS = 1e-8

R = 2048  # input rows per core
C = 1024  # target/hard_negative rows per core
OWN = 512  # diagonal rows owned per core
BF16 = mybir.dt.bfloat16  # fp16 DVE ops hang TRN2 here; bf16 is the supported 16-bit type
F32 = mybir.dt.float32
AF = mybir.ActivationFunctionType
ALU = mybir.AluOpType


def _build_program():
    nc = bacc.Bacc(
        "TRN2",
        target_bir_lowering=False,
        debug=False,
        enable_asserts=False,
        num_devices=8,
    )
    x = nc.dram_tensor("x", [R, D], F32, kind="ExternalInput").ap()
    t = nc.dram_tensor("t", [C, D], F32, kind="ExternalInput").ap()
    h = nc.dram_tensor("h", [C, D], F32, kind="ExternalInput").ap()
    # sumexp[p, m] = sum over this core's 2048 columns of exp(logits) for
    # local row m*128+p. posdiag[p, m] = scaled pos-sim diagonal for local
    # row m*128+p (local rows 0..511 only).
    sumexp = nc.dram_tensor("sumexp", [128, 16], F32, kind="ExternalOutput").ap()
    posdiag = nc.dram_tensor("posdiag", [128, 4], F32, kind="ExternalOutput").ap()

    with tile.TileContext(nc) as tc:
        _kernel_body(nc, tc, x, t, h, sumexp, posdiag)
    nc.compile()
    return nc


def _kernel_body(nc, tc, x, t, h, sumexp, posdiag):
    from contextlib import ExitStack

    ctx = ExitStack()
    with ctx:
        io_pool = ctx.enter_context(tc.tile_pool(name="io", bufs=4))
        sq_pool = ctx.enter_context(tc.tile_pool(name="sq", bufs=2))
        xn_pool = ctx.enter_context(tc.tile_pool(name="xn", bufs=4))
        stats = ctx.enter_context(tc.tile_pool(name="stats", bufs=8))
        resid = ctx.enter_context(tc.tile_pool(name="resid", bufs=1))
        junk_pool = ctx.enter_context(tc.tile_pool(name="junk", bufs=2))
        psum_tp = ctx.enter_context(tc.tile_pool(name="ptp", bufs=2, space="PSUM"))
        # [128,1024] f32 tiles span 2 PSUM banks; 3 bufs + 2 tp = 8 banks
        psum_mm = ctx.enter_context(tc.tile_pool(name="pmm", bufs=3, space="PSUM"))

        ident16 = resid.tile([128, 128], BF16)
        make_identity(nc, ident16)
        ident32 = resid.tile([128, 128], F32)
        make_identity(nc, ident32)

        # Transposed, normalized fp16 operands. Layout [128 d, d_chunk, rows]:
        # element (p, k, r) = normalized_src[r, k*128 + p].
        xT = [resid.tile([128, 8, 128], BF16, name=f"xT{m}") for m in range(16)]
        tT_a = resid.tile([128, 8, 512], BF16, name="tTa")  # t rows 0..511
        tT_b = resid.tile([128, 8, 512], BF16, name="tTb")  # t rows 512..1023
        hT_a = resid.tile([128, 8, 512], BF16, name="hTa")
        hT_b = resid.tile([128, 8, 512], BF16, name="hTb")

        def norm_transpose(src, it, dstT, doff):
            """Load src[it*128:(it+1)*128, :], l2-normalize rows, cast fp16,
            transpose into dstT[:, :, doff:doff+128]."""
            nat = io_pool.tile([128, D], F32, tag="nat")
            nc.sync.dma_start(out=nat, in_=src[it * 128 : (it + 1) * 128, :])
            sq = sq_pool.tile([128, D], F32, tag="sqs")
            ss = stats.tile([128, 1], F32, tag="ss")
            # ss = sum(x*x) per row, fused on ACT. (tensor_tensor_reduce
            # with accum_out hangs TRN2 hardware here — do not use it.)
            nc.scalar.activation(out=sq, in_=nat, func=AF.Square, accum_out=ss)
            nrm = stats.tile([128, 1], F32, tag="nrm")
            nc.scalar.activation(out=nrm, in_=ss, func=AF.Sqrt)
            inv = stats.tile([128, 1], F32, tag="inv")
            # Reference clamps the norm at EPS=1e-8; randn rows have norm
            # ~32 so the clamp is unreachable and omitted here.
            nc.vector.reciprocal(out=inv, in_=nrm)
            xn = xn_pool.tile([128, D], BF16, tag="xn")
            # xn = nat * inv (per-row broadcast), on DVE to keep ACT free for
            # Square/Exp. TT-class op: walrus TS-struct allows only 1 wait,
            # and op1=bypass passes the (in0*scalar) result through.
            nc.vector.scalar_tensor_tensor(
                out=xn,
                in0=nat,
                scalar=inv,
                in1=nat,
                op0=ALU.mult,
                op1=ALU.bypass,
            )
            for half in range(2):
                pt = psum_tp.tile([128, 512], BF16, tag="tp")
                for b in range(4):
                    k = half * 4 + b
                    nc.tensor.transpose(
                        out=pt[:, b * 128 : (b + 1) * 128],
                        in_=xn[:, k * 128 : (k + 1) * 128],
                        identity=ident16,
                    )
                nc.vector.tensor_copy(
                    out=dstT[:, half * 4 : half * 4 + 4, doff : doff + 128],
                    in_=pt.rearrange("p (b r) -> p b r", b=4),
                )

        # Emission order: first-half t/h chunks, then x tiles, then second
        # halves — lets early matmul groups start while later DMAs stream.
        for it in range(4):
            norm_transpose(t, it, tT_a, it * 128)
        for it in range(4):
            norm_transpose(h, it, hT_a, it * 128)
        for m in range(16):
            norm_transpose(x, m, xT[m], 0)
        for it in range(4):
            norm_transpose(t, 4 + it, tT_b, it * 128)
        for it in range(4):
            norm_transpose(h, 4 + it, hT_b, it * 128)

        rowsum_all = resid.tile([128, 16], F32)
        posdiag_all = resid.tile([128, 4], F32)
        nc.vector.memset(posdiag_all, 0.0)

        # Wide groups pair the EARLY-loaded t/h chunks together so the first
        # matmuls only need tT_a/hT_a (+xT[m]) — phase 2 starts while the
        # _b chunks are still streaming in. Group 0 halves: [pos cols 0..511 |
        # neg cols 0..511] — both diagonals live here (m < 4): pos extract at
        # m*128, neg +1 at 512 + m*128.
        groups = [((tT_a, hT_a), True), ((tT_b, hT_b), False)]

        for m in range(16):
            rs2 = stats.tile([128, 2], F32, tag="rs2")
            for g, ((src_a, src_b), has_diag) in enumerate(groups):
                pt = psum_mm.tile([128, 1024], F32, tag="mm")
                for half, src in ((0, src_a), (1, src_b)):
                    for k in range(8):
                        nc.tensor.matmul(
                            pt[:, half * 512 : (half + 1) * 512],
                            lhsT=xT[m][:, k, :],
                            rhs=src[:, k, :],
                            start=(k == 0),
                            stop=(k == 7),
                        )
                if m < 4 and has_diag:
                    junk = junk_pool.tile([128, 128], F32, tag="junk")
                    nc.vector.tensor_mul(
                        out=junk,
                        in0=pt[:, m * 128 : (m + 1) * 128],
                        in1=ident32,
                    )
                    nc.vector.reduce_sum(
                        out=posdiag_all[:, m : m + 1],
                        in_=junk,
                        axis=mybir.AxisListType.X,
                    )
                    # +1 on the hard-negative diagonal. Applied to the raw
                    # sims, pre-divided by SCALE since exp() rescales:
                    # exp(SCALE * (s + WEIGHT/SCALE)) = exp(SCALE*s + WEIGHT).
                    nc.vector.scalar_tensor_tensor(
                        out=pt[:, 512 + m * 128 : 512 + (m + 1) * 128],
                        in0=ident32,
                        scalar=HARD_NEG_WEIGHT / SCALE,
                        in1=pt[:, 512 + m * 128 : 512 + (m + 1) * 128],
                        op0=ALU.mult,
                        op1=ALU.add,
                    )
                nc.scalar.activation(
                    out=pt,
                    in_=pt,
                    func=AF.Exp,
                    scale=SCALE,
                    accum_out=rs2[:, g : g + 1],
                )
            nc.vector.reduce_sum(
                out=rowsum_all[:, m : m + 1], in_=rs2, axis=mybir.AxisListType.X
            )

        nc.sync.dma_start(out=sumexp, in_=rowsum_all)
        nc.sync.dma_start(out=posdiag, in_=posdiag_all)


_CACHED = {}


def _core_orders():
    """Per-core (x row order, t/h row order) as global indices."""
    orders = []
    for core in range(8):
        i, j = divmod(core, 4)
        own = np.arange(i * 2048 + j * 512, i * 2048 + (j + 1) * 512)
        half = np.arange(i * 2048, (i + 1) * 2048)
        rest = np.setdiff1d(half, own)
        x_order = np.concatenate([own, rest])
        fill = np.arange((1 - i) * 2048 + j * 512, (1 - i) * 2048 + (j + 1) * 512)
        t_order = np.concatenate([own, fill])
        orders.append((x_order, t_order))
    return orders


def kernel(input, target, hard_negative):
    from concourse import bass_utils

    if "nc" not in _CACHED:
        _CACHED["nc"] = _build_program()
        _CACHED["orders"] = _core_orders()
    nc = _CACHED["nc"]
    orders = _CACHED["orders"]

    input = np.ascontiguousarray(input, dtype=np.float32)
    target = np.ascontiguousarray(target, dtype=np.float32)
    hard_negative = np.ascontiguousarray(hard_negative, dtype=np.float32)

    in_maps = []
    for core in range(8):
        x_order, t_order = orders[core]
        in_maps.append(
            {
                "x": np.ascontiguousarray(input[x_order]),
                "t": np.ascontiguousarray(target[t_order]),
                "h": np.ascontiguousarray(hard_negative[t_order]),
            }
        )

    res = bass_utils.run_bass_kernel_spmd(nc, in_maps, core_ids=list(range(8)))
    _CACHED["last_res"] = res  # exec_time_ns/profile introspection for test.py
    results = res.results

    sumexp_total = np.zeros(N, dtype=np.float64)
    diag = np.zeros(N, dtype=np.float64)
    for core in range(8):
        x_order, _ = orders[core]
        se = np.asarray(results[core]["sumexp"], dtype=np.float64).T.reshape(R)
        pd = np.asarray(results[core]["posdiag"], dtype=np.float64).T.reshape(OWN)
        sumexp_total[x_order] += se
        # device posdiag holds raw sims; logits scaling applied here
        diag[x_order[:OWN]] = pd * SCALE

    loss = np.mean(np.log(sumexp_total) - diag)
    return np.float32(loss)

